# revision 19
# baseline (speedup 1.0000x reference)
"""Trainium2 Bass kernel for nn_CheapChannelV1 (dense_cnn).

Strategy (per core, pure data-parallel over batch):
  - The three channel-shuffle + 1x1-conv stages are linear, so they fold on the
    host into ONE 128x128 matrix M and bias b_tot:  res3 = M @ s + b_tot, where
    s = [s0;s1;s2;s3] are the four depthwise-conv branch outputs.
  - Level-0 depthwise conv (full res) folds INTO the matmul: 9 tap matmuls
    (K=32) reading shifted views of a zero-padded x0 strip tile.
  - Levels 1-3: max-pool on DVE (strided TT max), 3x3 depthwise conv on DVE
    (fp32 accum); nearest-upsample folds into broadcast rhs APs of the group
    matmuls.
  - 12 accumulating K=32 matmuls per 512-px chunk, spread across the four PE
    row groups via tile_position for quadrant concurrency; x0 taps first,
    conv-group matmuls last, so the PE starts a band before its convs finish.
  - Whole datapath bf16 (PSUM + conv accum fp32): fp32 matmuls are
    LDWEIGHTS-bound and 2-pass; bf16 is ~4x on the PE, 2x DMA.
  - Two-band-deep pipeline: band b is pooled at iteration b, its convs run at
    b+1, its matmuls/epilogue at b+2 — the DVE phase of one band overlaps the
    PE/ACT/GPSIMD phase of the previous one.
  - Epilogue: exact Gelu on ACT (bias folded in), multiply-by-x on GPSIMD
    (DVE helps near the tail), 16-row batched output DMAs from the Scalar
    queue.  x0 loads also issue from Scalar so the in-order Sync queue's
    scatter waits never delay them.
"""

import numpy as np

H = W = 256
CH = 128
BANDS = [(0, 16), (16, 16), (32, 16), (48, 16)]
NB = len(BANDS)


def _shuf_cols(A, groups=8):
    # Returns A' with A' @ s == A @ channel_shuffle(s)
    Cin = A.shape[1]
    idx = np.arange(Cin)
    perm = (idx % groups) * (Cin // groups) + idx // groups
    Ap = np.zeros_like(A)
    Ap[:, perm] = A
    return Ap


def fold_weights(w_dw, b_dw, w_f1, b_f1, w_f2, b_f2, w_f3, b_f3):
    f8 = np.float64
    A1 = _shuf_cols(w_f1.astype(f8))
    A2 = _shuf_cols(w_f2.astype(f8))
    A3 = _shuf_cols(w_f3.astype(f8))
    A2a, A2b = A2[:, :64], A2[:, 64:]
    A3a, A3b = A3[:, :96], A3[:, 96:]
    M = np.zeros((128, 128), f8)
    M[:, 0:64] = A3a @ A2a @ A1
    M[:, 64:96] = A3a @ A2b
    M[:, 96:128] = A3b
    b_tot = A3a @ (A2a @ b_f1.astype(f8) + b_f2.astype(f8)) + b_f3.astype(f8)
    for g in range(4):
        b_tot = b_tot + M[:, 32 * g:32 * g + 32] @ b_dw[g].astype(f8)

    # W_all[p, t, o]: lhsT matrices, identical content per 32-partition group.
    W_all = np.zeros((128, 12, 128), np.float32)
    M0T = M[:, 0:32].T          # [32(c), 128(o)]
    w0 = w_dw[0].reshape(32, 9).astype(f8)
    for gp in range(4):
        rows = slice(32 * gp, 32 * gp + 32)
        for j in range(9):
            W_all[rows, j, :] = (M0T * w0[:, j:j + 1]).astype(np.float32)
        W_all[rows, 9, :] = M[:, 32:64].T.astype(np.float32)
        W_all[rows, 10, :] = M[:, 64:96].T.astype(np.float32)
        W_all[rows, 11, :] = M[:, 96:128].T.astype(np.float32)

    wdwp = np.zeros((128, 3, 9), np.float32)
    for g in (1, 2, 3):
        wdwp[:, g - 1, :] = np.tile(w_dw[g].reshape(32, 9), (4, 1)).astype(np.float32)

    return W_all, b_tot.astype(np.float32).reshape(128, 1), wdwp


_PROGRAM_CACHE = {}


def build_program(act_func_name="Gelu"):
    key = act_func_name
    if key in _PROGRAM_CACHE:
        return _PROGRAM_CACHE[key]

    import concourse.bacc as bacc
    import concourse.tile as tile
    import concourse.mybir as mybir

    f32 = mybir.dt.float32
    bf16 = mybir.dt.bfloat16
    AOT = mybir.AluOpType
    act_func = getattr(mybir.ActivationFunctionType, act_func_name)

    nc = bacc.Bacc("TRN2", target_bir_lowering=False, debug=False)
    x_d = nc.dram_tensor("x", [CH, H, W], bf16, kind="ExternalInput")
    wall_d = nc.dram_tensor("wall", [128, 12, 128], bf16, kind="ExternalInput")
    btot_d = nc.dram_tensor("btot", [128, 1], f32, kind="ExternalInput")
    wdwp_d = nc.dram_tensor("wdwp", [128, 3, 9], f32, kind="ExternalInput")
    out_d = nc.dram_tensor("out", [CH, H, W], bf16, kind="ExternalOutput")

    # x viewed as [128, block r, row-in-block, col]
    x_blk = x_d[:].rearrange("p (r hh) w -> p r hh w", r=4)

    with tile.TileContext(nc) as tc:
        with tc.tile_pool(name="persist", bufs=1) as pers, \
             tc.tile_pool(name="xband", bufs=3) as xpool, \
             tc.tile_pool(name="x0strip", bufs=2) as x0pool, \
             tc.tile_pool(name="ptmp", bufs=1) as ptmp, \
             tc.tile_pool(name="convb", bufs=2) as cpool, \
             tc.tile_pool(name="psum", bufs=8, space="PSUM") as pspool, \
             tc.tile_pool(name="gout", bufs=4) as gpool, \
             tc.tile_pool(name="mout", bufs=4) as mpool:

            # --- persistent weights / strips -----------------------------
            wall = pers.tile([128, 12, 128], bf16)
            nc.sync.dma_start(wall[:], wall_d[:])
            btot = pers.tile([128, 1], f32)
            nc.sync.dma_start(btot[:], btot_d[:])
            wdwp = pers.tile([128, 3, 9], f32)
            nc.sync.dma_start(wdwp[:], wdwp_d[:])

            p1pad = pers.tile([128, 34, 130], bf16)
            p2pad = pers.tile([128, 18, 66], bf16)
            p3pad = pers.tile([128, 10, 34], bf16)
            nc.gpsimd.memset(p1pad[:], 0.0)
            nc.gpsimd.memset(p2pad[:], 0.0)
            nc.gpsimd.memset(p3pad[:], 0.0)

            # Top halos for pooled strips: strip rho's first conv row needs
            # the last pooled row of block rho-1, which only streams in at the
            # last band. Pool it up-front from a redundant load of the 8 image
            # rows preceding each block (r=1,2,3).
            xh = xpool.tile([128, 3, 8, 256], bf16, tag="xb")
            nc.sync.dma_start(xh[:], x_blk[:, 0:3, 56:64, :])
            hh1 = ptmp.tile([128, 3, 8, 128], bf16, tag="hp1")
            nc.vector.tensor_tensor(
                hh1[:], xh[:, :, :, 0::2], xh[:, :, :, 1::2], AOT.max)
            hv1 = ptmp.tile([128, 3, 4, 128], bf16, tag="vp1")
            nc.vector.tensor_tensor(
                hv1[:], hh1[:, :, 0::2, :], hh1[:, :, 1::2, :], AOT.max)
            hh2 = ptmp.tile([128, 3, 4, 64], bf16, tag="hp2")
            nc.vector.tensor_tensor(
                hh2[:], hv1[:, :, :, 0::2], hv1[:, :, :, 1::2], AOT.max)
            hv2 = ptmp.tile([128, 3, 2, 64], bf16, tag="vp2")
            nc.vector.tensor_tensor(
                hv2[:], hh2[:, :, 0::2, :], hh2[:, :, 1::2, :], AOT.max)
            hh3 = ptmp.tile([128, 3, 2, 32], bf16, tag="hp3")
            nc.vector.tensor_tensor(
                hh3[:], hv2[:, :, :, 0::2], hv2[:, :, :, 1::2], AOT.max)
            hv3 = ptmp.tile([128, 3, 1, 32], bf16, tag="vp3")
            nc.vector.tensor_tensor(
                hv3[:], hh3[:, :, 0::2, :], hh3[:, :, 1::2, :], AOT.max)
            for r in (1, 2, 3):
                g = r * 32
                nc.sync.dma_start(p1pad[g:g + 32, 0:1, 1:129],
                                  hv1[32:64, r - 1, 3:4, :])
                nc.sync.dma_start(p2pad[g:g + 32, 0:1, 1:65],
                                  hv2[64:96, r - 1, 1:2, :])
                nc.sync.dma_start(p3pad[g:g + 32, 0:1, 1:33],
                                  hv3[96:128, r - 1, 0:1, :])

            xbands, x0s, convs = {}, {}, {}

            for it in range(NB + 2):
                # ------------- load + pool phase (band b = it) ---------------
                if it < NB:
                    b = it
                    s_, n_ = BANDS[b]
                    n2, n4, n8 = n_ // 2, n_ // 4, n_ // 8
                    xband = xpool.tile([128, 4, n_, 256], bf16, tag="xb")
                    xbands[b] = xband
                    nc.sync.dma_start(
                        xband[:], x_blk[:, :, s_: s_ + n_, :])

                # x0 strip loads: issued on sync right after the xband
                # load, BEFORE the scatter DMAs, so the in-order sync queue
                # never parks them behind a pooling-dependent wait.  Band 0's
                # strip loads at iteration 0 (ahead of scatter(0)) so the
                # first tap matmuls can start immediately.
                bx = 0 if it == 0 else (it - 1 if it >= 2 else None)
                if bx is not None and bx <= NB - 1:
                    bc = bx
                    sc, ncr = BANDS[bc]
                    c2n, c4n, c8n = ncr // 2, ncr // 4, ncr // 8
                    x0 = x0pool.tile([128, ncr + 2, 258], bf16)
                    x0s[bc] = x0
                    # zero the left/right pad columns (0 and 257)
                    nc.vector.memset(x0[:, :, ::257], 0.0)
                    for r in range(4):
                        lo = 64 * r + sc - 1
                        hi = lo + ncr + 2
                        dlo, dhi = 0, ncr + 2
                        if lo < 0:
                            nc.vector.memset(x0[32 * r:32 * r + 32, 0, :], 0.0)
                            dlo, lo = 1, 0
                        if hi > 256:
                            nc.vector.memset(
                                x0[32 * r:32 * r + 32, ncr + 1, :], 0.0)
                            dhi, hi = ncr + 1, 256
                        nc.sync.dma_start(
                            x0[32 * r:32 * r + 32, dlo:dhi, 1:257],
                            x_d[0:32, lo:hi, :])

                if it < NB:
                    # hierarchical 2x2 max pooling (channels 32..127);
                    # full-128-partition ops (lanes for unused channel groups
                    # compute junk for free; only the real slices get read).
                    hp1 = ptmp.tile([128, 4, n_, 128], bf16, tag="hp1")
                    nc.vector.tensor_tensor(
                        hp1[:], xband[:, :, :, 0::2],
                        xband[:, :, :, 1::2], AOT.max)
                    vp1 = ptmp.tile([128, 4, n2, 128], bf16, tag="vp1")
                    nc.vector.tensor_tensor(
                        vp1[:], hp1[:, :, 0::2, :],
                        hp1[:, :, 1::2, :], AOT.max)
                    hp2 = ptmp.tile([128, 4, n2, 64], bf16, tag="hp2")
                    nc.vector.tensor_tensor(
                        hp2[:], vp1[:, :, :, 0::2],
                        vp1[:, :, :, 1::2], AOT.max)
                    vp2 = ptmp.tile([128, 4, n4, 64], bf16, tag="vp2")
                    nc.vector.tensor_tensor(
                        vp2[:], hp2[:, :, 0::2, :],
                        hp2[:, :, 1::2, :], AOT.max)
                    hp3 = ptmp.tile([128, 4, n4, 32], bf16, tag="hp3")
                    nc.vector.tensor_tensor(
                        hp3[:], vp2[:, :, :, 0::2],
                        vp2[:, :, :, 1::2], AOT.max)
                    vp3 = ptmp.tile([128, 4, n8, 32], bf16, tag="vp3")
                    nc.vector.tensor_tensor(
                        vp3[:], hp3[:, :, 0::2, :],
                        hp3[:, :, 1::2, :], AOT.max)

                    # scatter into persistent padded strip buffers
                    for r in range(4):
                        g0 = r * 32
                        nc.sync.dma_start(
                            p1pad[g0:g0 + 32,
                                  s_ // 2 + 1:s_ // 2 + 1 + n2, 1:129],
                            vp1[32:64, r])
                        nc.sync.dma_start(
                            p2pad[g0:g0 + 32,
                                  s_ // 4 + 1:s_ // 4 + 1 + n4, 1:65],
                            vp2[64:96, r])
                        nc.sync.dma_start(
                            p3pad[g0:g0 + 32,
                                  s_ // 8 + 1:s_ // 8 + 1 + n8, 1:33],
                            vp3[96:128, r])
                        if b == 0 and r > 0:   # bottom halos of strip r-1
                            gm = (r - 1) * 32
                            nc.sync.dma_start(
                                p1pad[gm:gm + 32, 33:34, 1:129],
                                vp1[32:64, r, 0:1, :])
                            nc.sync.dma_start(
                                p2pad[gm:gm + 32, 17:18, 1:65],
                                vp2[64:96, r, 0:1, :])
                            nc.sync.dma_start(
                                p3pad[gm:gm + 32, 9:10, 1:33],
                                vp3[96:128, r, 0:1, :])

                # ------------- pooled convs (band bc = it-1) -----------------
                if 1 <= it <= NB:
                    # pooled convs (all 4 strips at once); fp32 accumulation,
                    # final tap casts to bf16 for the matmul rhs.
                    c1f = ptmp.tile([128, c2n, 128], f32, tag="c1f")
                    c2f = ptmp.tile([128, c4n, 64], f32, tag="c2f")
                    c3f = ptmp.tile([128, c8n, 32], f32, tag="c3f")
                    conv1 = cpool.tile([128, c2n, 128], bf16, tag="c1b")
                    conv2 = cpool.tile([128, c4n, 64], bf16, tag="c2b")
                    conv3 = cpool.tile([128, c8n, 32], bf16, tag="c3b")
                    convs[bc] = (conv1, conv2, conv3)
                    for j in range(9):
                        dy, dx = j // 3, j % 3
                        a1 = p1pad[:, sc // 2 + dy:sc // 2 + dy + c2n,
                                   dx:dx + 128]
                        a2 = p2pad[:, sc // 4 + dy:sc // 4 + dy + c4n,
                                   dx:dx + 64]
                        a3 = p3pad[:, sc // 8 + dy:sc // 8 + dy + c8n,
                                   dx:dx + 32]
                        if j == 0:
                            nc.vector.tensor_scalar_mul(
                                c1f[:], a1, wdwp[:, 0, 0:1])
                            nc.vector.tensor_scalar_mul(
                                c2f[:], a2, wdwp[:, 1, 0:1])
                            nc.vector.tensor_scalar_mul(
                                c3f[:], a3, wdwp[:, 2, 0:1])
                        else:
                            o1, o2, o3 = ((conv1, conv2, conv3) if j == 8
                                          else (c1f, c2f, c3f))
                            nc.vector.scalar_tensor_tensor(
                                o1[:], a1, wdwp[:, 0, j:j + 1], c1f[:],
                                AOT.mult, AOT.add)
                            nc.vector.scalar_tensor_tensor(
                                o2[:], a2, wdwp[:, 1, j:j + 1], c2f[:],
                                AOT.mult, AOT.add)
                            nc.vector.scalar_tensor_tensor(
                                o3[:], a3, wdwp[:, 2, j:j + 1], c3f[:],
                                AOT.mult, AOT.add)

                # ------------- matmuls + epilogue (band bb = it-2) -----------
                if it >= 2:
                    bb = it - 2
                    sb_, nb_ = BANDS[bb]
                    conv1, conv2, conv3 = convs.pop(bb)
                    x0b = x0s.pop(bb)
                    xbb = xbands.pop(bb)
                    # multiply-engine split: DVE helps at the tail when it has
                    # no more pooling/conv work; GPSIMD carries the rest.
                    dve_rs = (0, 1) if bb == NB - 1 else \
                        ((0,) if bb >= 1 else ())
                    mts = [mpool.tile([128, nb_, 256], bf16, tag="mchunk",
                                      name=f"mt_{bb}_{r}")
                           for r in range(4)]
                    for pg in range(nb_ // 4):
                        pss = [[pspool.tile([128, 2, 256], f32, tag="pschunk",
                                            name=f"ps_{bb}_{pg}_{r}_{ic}")
                                for ic in range(2)] for r in range(4)]
                        # taps first (need only x0), conv groups last — so
                        # the PE starts a band's chunks while DVE finishes
                        # that band's pooled convs.  The two chunks of a pair
                        # share each LDWEIGHTS (second matmul reuses the
                        # stationary weights).
                        for tt in range(12):
                            t = tt + 3 if tt < 9 else tt - 9
                            for r in range(4):
                                g0 = 32 * r
                                if t < 3:
                                    lhsT = wall[g0:g0 + 32, 9 + t, :]
                                else:
                                    lhsT = wall[g0:g0 + 32, t - 3, :]
                                for ic in range(2):
                                    i = 2 * pg + ic
                                    if t == 0:
                                        rhs = conv1[g0:g0 + 32, i, :] \
                                            .unsqueeze(1).unsqueeze(3) \
                                            .broadcast_to([32, 2, 128, 2])
                                    elif t == 1:
                                        i2 = (sb_ // 2 + i) // 2 - sb_ // 4
                                        rhs = conv2[g0:g0 + 32, i2, :] \
                                            .unsqueeze(1).unsqueeze(3) \
                                            .broadcast_to([32, 2, 64, 4])
                                    elif t == 2:
                                        i3 = (sb_ // 2 + i) // 4 - sb_ // 8
                                        rhs = conv3[g0:g0 + 32, i3, :] \
                                            .unsqueeze(1).unsqueeze(3) \
                                            .broadcast_to([32, 2, 32, 8])
                                    else:
                                        j = t - 3
                                        dy, dx = j // 3, j % 3
                                        rhs = x0b[g0:g0 + 32,
                                                  2 * i + dy:2 * i + dy + 2,
                                                  dx:dx + 256]
                                    mm = nc.tensor.matmul(
                                        pss[r][ic][:], lhsT, rhs,
                                        start=(tt == 0), stop=(tt == 11),
                                        tile_position=(g0, 0))
                                    if ic == 1:
                                        mm.ldweights = False
                        for r in range(4):
                            for ic in range(2):
                                i = 2 * pg + ic
                                gt = gpool.tile([128, 2, 256], bf16,
                                                tag="gchunk")
                                nc.scalar.activation(
                                    gt[:], pss[r][ic][:], act_func,
                                    bias=btot[:, 0:1])
                                ms = mts[r][:, 2 * i:2 * i + 2, :]
                                xs = xbb[:, r, 2 * i:2 * i + 2, :]
                                if r in dve_rs:
                                    nc.vector.tensor_tensor(
                                        ms, gt[:], xs, AOT.mult)
                                else:
                                    nc.gpsimd.tensor_mul(ms, gt[:], xs)
                    for r in range(4):
                        nc.scalar.dma_start(
                            out_d[:, 64 * r + sb_: 64 * r + sb_ + nb_, :],
                            mts[r][:])

    nc.compile()
    _PROGRAM_CACHE[key] = nc
    return nc


def make_in_maps(x, w_dw, b_dw, w_f1, b_f1, w_f2, b_f2, w_f3, b_f3):
    import ml_dtypes
    bf = ml_dtypes.bfloat16
    x = np.asarray(x)
    B = x.shape[0]
    W_all, b_tot, wdwp = fold_weights(
        np.asarray(w_dw), np.asarray(b_dw), np.asarray(w_f1), np.asarray(b_f1),
        np.asarray(w_f2), np.asarray(b_f2), np.asarray(w_f3), np.asarray(b_f3))
    wall_b = np.ascontiguousarray(W_all.astype(bf))
    in_maps = [{"x": np.ascontiguousarray(x[i].astype(bf)),
                "wall": wall_b, "btot": b_tot, "wdwp": wdwp}
               for i in range(B)]
    return in_maps


def kernel(x, w_dw, b_dw, w_f1, b_f1, w_f2, b_f2, w_f3, b_f3):
    from concourse.bass_utils import run_bass_kernel_spmd

    x = np.asarray(x)
    B = x.shape[0]
    in_maps = make_in_maps(x, w_dw, b_dw, w_f1, b_f1, w_f2, b_f2, w_f3, b_f3)
    nc = build_program("Gelu")
    res = run_bass_kernel_spmd(nc, in_maps, list(range(B)))
    out = np.stack([res.results[i]["out"] for i in range(B)], axis=0)
    return out.astype(np.float32)


# revision 20
# speedup vs baseline: 1.2184x; 1.2184x over previous
"""Trainium2 Bass kernel for nn_CheapChannelV1 (dense_cnn).

Strategy (per core, pure data-parallel over batch):
  - The three channel-shuffle + 1x1-conv stages are linear, so they fold on the
    host into ONE 128x128 matrix M and bias b_tot:  res3 = M @ s + b_tot, where
    s = [s0;s1;s2;s3] are the four depthwise-conv branch outputs.
  - Level-0 depthwise conv (full res) folds INTO the matmul: 9 tap matmuls
    (K=32) reading shifted views of a zero-padded x0 strip tile.
  - Levels 1-3: max-pool on DVE (strided TT max), 3x3 depthwise conv on DVE
    (fp32 accum); nearest-upsample folds into broadcast rhs APs of the group
    matmuls.
  - 12 accumulating K=32 matmuls per 512-px chunk, spread across the four PE
    row groups via tile_position for quadrant concurrency; x0 taps first,
    conv-group matmuls last, so the PE starts a band before its convs finish.
  - Whole datapath bf16 (PSUM + conv accum fp32): fp32 matmuls are
    LDWEIGHTS-bound and 2-pass; bf16 is ~4x on the PE, 2x DMA.
  - Two-band-deep pipeline: band b is pooled at iteration b, its convs run at
    b+1, its matmuls/epilogue at b+2 — the DVE phase of one band overlaps the
    PE/ACT/GPSIMD phase of the previous one.
  - Epilogue: exact Gelu on ACT (bias folded in), multiply-by-x on GPSIMD
    (DVE helps near the tail), 16-row batched output DMAs from the Scalar
    queue.  x0 loads also issue from Scalar so the in-order Sync queue's
    scatter waits never delay them.
"""

import numpy as np

H = W = 256
CH = 128
BANDS = [(0, 16), (16, 16), (32, 16), (48, 16)]
NB = len(BANDS)


def _shuf_cols(A, groups=8):
    # Returns A' with A' @ s == A @ channel_shuffle(s)
    Cin = A.shape[1]
    idx = np.arange(Cin)
    perm = (idx % groups) * (Cin // groups) + idx // groups
    Ap = np.zeros_like(A)
    Ap[:, perm] = A
    return Ap


def fold_weights(w_dw, b_dw, w_f1, b_f1, w_f2, b_f2, w_f3, b_f3):
    f8 = np.float64
    A1 = _shuf_cols(w_f1.astype(f8))
    A2 = _shuf_cols(w_f2.astype(f8))
    A3 = _shuf_cols(w_f3.astype(f8))
    A2a, A2b = A2[:, :64], A2[:, 64:]
    A3a, A3b = A3[:, :96], A3[:, 96:]
    M = np.zeros((128, 128), f8)
    M[:, 0:64] = A3a @ A2a @ A1
    M[:, 64:96] = A3a @ A2b
    M[:, 96:128] = A3b
    b_tot = A3a @ (A2a @ b_f1.astype(f8) + b_f2.astype(f8)) + b_f3.astype(f8)
    for g in range(4):
        b_tot = b_tot + M[:, 32 * g:32 * g + 32] @ b_dw[g].astype(f8)

    # W_all[p, t, o]: lhsT matrices, identical content per 32-partition group.
    W_all = np.zeros((128, 12, 128), np.float32)
    M0T = M[:, 0:32].T          # [32(c), 128(o)]
    w0 = w_dw[0].reshape(32, 9).astype(f8)
    for gp in range(4):
        rows = slice(32 * gp, 32 * gp + 32)
        for j in range(9):
            W_all[rows, j, :] = (M0T * w0[:, j:j + 1]).astype(np.float32)
        W_all[rows, 9, :] = M[:, 32:64].T.astype(np.float32)
        W_all[rows, 10, :] = M[:, 64:96].T.astype(np.float32)
        W_all[rows, 11, :] = M[:, 96:128].T.astype(np.float32)

    wdwp = np.zeros((128, 3, 9), np.float32)
    for g in (1, 2, 3):
        wdwp[:, g - 1, :] = np.tile(w_dw[g].reshape(32, 9), (4, 1)).astype(np.float32)

    return W_all, b_tot.astype(np.float32).reshape(128, 1), wdwp


_PROGRAM_CACHE = {}


def build_program(act_func_name="Gelu"):
    key = act_func_name
    if key in _PROGRAM_CACHE:
        return _PROGRAM_CACHE[key]

    import concourse.bacc as bacc
    import concourse.tile as tile
    import concourse.mybir as mybir

    f32 = mybir.dt.float32
    bf16 = mybir.dt.bfloat16
    AOT = mybir.AluOpType
    act_func = getattr(mybir.ActivationFunctionType, act_func_name)

    nc = bacc.Bacc("TRN2", target_bir_lowering=False, debug=False)
    x_d = nc.dram_tensor("x", [CH, H, W], bf16, kind="ExternalInput")
    wall_d = nc.dram_tensor("wall", [128, 12, 128], bf16, kind="ExternalInput")
    btot_d = nc.dram_tensor("btot", [128, 1], f32, kind="ExternalInput")
    wdwp_d = nc.dram_tensor("wdwp", [128, 3, 9], f32, kind="ExternalInput")
    out_d = nc.dram_tensor("out", [CH, H, W], bf16, kind="ExternalOutput")

    # x viewed as [128, block r, row-in-block, col]
    x_blk = x_d[:].rearrange("p (r hh) w -> p r hh w", r=4)

    with tile.TileContext(nc) as tc:
        with tc.tile_pool(name="persist", bufs=1) as pers, \
             tc.tile_pool(name="xband", bufs=3) as xpool, \
             tc.tile_pool(name="x0strip", bufs=2) as x0pool, \
             tc.tile_pool(name="ptmp", bufs=1) as ptmp, \
             tc.tile_pool(name="convb", bufs=2) as cpool, \
             tc.tile_pool(name="psum", bufs=8, space="PSUM") as pspool, \
             tc.tile_pool(name="gout", bufs=4) as gpool, \
             tc.tile_pool(name="mout", bufs=4) as mpool:

            # --- persistent weights / strips -----------------------------
            wall = pers.tile([128, 12, 128], bf16)
            nc.sync.dma_start(wall[:], wall_d[:])
            btot = pers.tile([128, 1], f32)
            nc.sync.dma_start(btot[:], btot_d[:])
            wdwp = pers.tile([128, 3, 9], f32)
            nc.sync.dma_start(wdwp[:], wdwp_d[:])

            p1pad = pers.tile([128, 34, 130], bf16)
            p2pad = pers.tile([128, 18, 66], bf16)
            p3pad = pers.tile([128, 10, 34], bf16)
            nc.gpsimd.memset(p1pad[:], 0.0)
            nc.gpsimd.memset(p2pad[:], 0.0)
            nc.gpsimd.memset(p3pad[:], 0.0)

            # Top halos for pooled strips: strip rho's first conv row needs
            # the last pooled row of block rho-1, which only streams in at the
            # last band. Pool it up-front from a redundant load of the 8 image
            # rows preceding each block (r=1,2,3).
            xh = xpool.tile([128, 3, 8, 256], bf16, tag="xb")
            nc.sync.dma_start(xh[:], x_blk[:, 0:3, 56:64, :])
            hh1 = ptmp.tile([128, 3, 8, 128], bf16, tag="hp1")
            nc.vector.tensor_tensor(
                hh1[:], xh[:, :, :, 0::2], xh[:, :, :, 1::2], AOT.max)
            hv1 = ptmp.tile([128, 3, 4, 128], bf16, tag="vp1")
            nc.vector.tensor_tensor(
                hv1[:], hh1[:, :, 0::2, :], hh1[:, :, 1::2, :], AOT.max)
            hh2 = ptmp.tile([128, 3, 4, 64], bf16, tag="hp2")
            nc.vector.tensor_tensor(
                hh2[:], hv1[:, :, :, 0::2], hv1[:, :, :, 1::2], AOT.max)
            hv2 = ptmp.tile([128, 3, 2, 64], bf16, tag="vp2")
            nc.vector.tensor_tensor(
                hv2[:], hh2[:, :, 0::2, :], hh2[:, :, 1::2, :], AOT.max)
            hh3 = ptmp.tile([128, 3, 2, 32], bf16, tag="hp3")
            nc.vector.tensor_tensor(
                hh3[:], hv2[:, :, :, 0::2], hv2[:, :, :, 1::2], AOT.max)
            hv3 = ptmp.tile([128, 3, 1, 32], bf16, tag="vp3")
            nc.vector.tensor_tensor(
                hv3[:], hh3[:, :, 0::2, :], hh3[:, :, 1::2, :], AOT.max)
            for r in (1, 2, 3):
                g = r * 32
                nc.sync.dma_start(p1pad[g:g + 32, 0:1, 1:129],
                                  hv1[32:64, r - 1, 3:4, :])
                nc.sync.dma_start(p2pad[g:g + 32, 0:1, 1:65],
                                  hv2[64:96, r - 1, 1:2, :])
                nc.sync.dma_start(p3pad[g:g + 32, 0:1, 1:33],
                                  hv3[96:128, r - 1, 0:1, :])

            xbands, x0s, convs = {}, {}, {}

            for it in range(NB + 2):
                # ------------- load + pool phase (band b = it) ---------------
                if it < NB:
                    b = it
                    s_, n_ = BANDS[b]
                    n2, n4, n8 = n_ // 2, n_ // 4, n_ // 8
                    xband = xpool.tile([128, 4, n_, 256], bf16, tag="xb")
                    xbands[b] = xband
                    nc.sync.dma_start(
                        xband[:], x_blk[:, :, s_: s_ + n_, :])

                # x0 strip loads: issued on sync right after the xband
                # load, BEFORE the scatter DMAs, so the in-order sync queue
                # never parks them behind a pooling-dependent wait.
                if 1 <= it <= NB:
                    bc = it - 1
                    sc, ncr = BANDS[bc]
                    c2n, c4n, c8n = ncr // 2, ncr // 4, ncr // 8
                    x0 = x0pool.tile([128, ncr + 2, 258], bf16)
                    x0s[bc] = x0
                    # zero the left/right pad columns (0 and 257)
                    nc.vector.memset(x0[:, :, ::257], 0.0)
                    for r in range(4):
                        lo = 64 * r + sc - 1
                        hi = lo + ncr + 2
                        dlo, dhi = 0, ncr + 2
                        if lo < 0:
                            nc.vector.memset(x0[32 * r:32 * r + 32, 0, :], 0.0)
                            dlo, lo = 1, 0
                        if hi > 256:
                            nc.vector.memset(
                                x0[32 * r:32 * r + 32, ncr + 1, :], 0.0)
                            dhi, hi = ncr + 1, 256
                        nc.sync.dma_start(
                            x0[32 * r:32 * r + 32, dlo:dhi, 1:257],
                            x_d[0:32, lo:hi, :])

                if it < NB:
                    # hierarchical 2x2 max pooling (channels 32..127);
                    # full-128-partition ops (lanes for unused channel groups
                    # compute junk for free; only the real slices get read).
                    hp1 = ptmp.tile([128, 4, n_, 128], bf16, tag="hp1")
                    nc.vector.tensor_tensor(
                        hp1[:], xband[:, :, :, 0::2],
                        xband[:, :, :, 1::2], AOT.max)
                    vp1 = ptmp.tile([128, 4, n2, 128], bf16, tag="vp1")
                    nc.vector.tensor_tensor(
                        vp1[:], hp1[:, :, 0::2, :],
                        hp1[:, :, 1::2, :], AOT.max)
                    hp2 = ptmp.tile([128, 4, n2, 64], bf16, tag="hp2")
                    nc.vector.tensor_tensor(
                        hp2[:], vp1[:, :, :, 0::2],
                        vp1[:, :, :, 1::2], AOT.max)
                    vp2 = ptmp.tile([128, 4, n4, 64], bf16, tag="vp2")
                    nc.vector.tensor_tensor(
                        vp2[:], hp2[:, :, 0::2, :],
                        hp2[:, :, 1::2, :], AOT.max)
                    hp3 = ptmp.tile([128, 4, n4, 32], bf16, tag="hp3")
                    nc.vector.tensor_tensor(
                        hp3[:], vp2[:, :, :, 0::2],
                        vp2[:, :, :, 1::2], AOT.max)
                    vp3 = ptmp.tile([128, 4, n8, 32], bf16, tag="vp3")
                    nc.vector.tensor_tensor(
                        vp3[:], hp3[:, :, 0::2, :],
                        hp3[:, :, 1::2, :], AOT.max)

                    # scatter into persistent padded strip buffers
                    for r in range(4):
                        g0 = r * 32
                        nc.sync.dma_start(
                            p1pad[g0:g0 + 32,
                                  s_ // 2 + 1:s_ // 2 + 1 + n2, 1:129],
                            vp1[32:64, r])
                        nc.sync.dma_start(
                            p2pad[g0:g0 + 32,
                                  s_ // 4 + 1:s_ // 4 + 1 + n4, 1:65],
                            vp2[64:96, r])
                        nc.sync.dma_start(
                            p3pad[g0:g0 + 32,
                                  s_ // 8 + 1:s_ // 8 + 1 + n8, 1:33],
                            vp3[96:128, r])
                        if b == 0 and r > 0:   # bottom halos of strip r-1
                            gm = (r - 1) * 32
                            nc.sync.dma_start(
                                p1pad[gm:gm + 32, 33:34, 1:129],
                                vp1[32:64, r, 0:1, :])
                            nc.sync.dma_start(
                                p2pad[gm:gm + 32, 17:18, 1:65],
                                vp2[64:96, r, 0:1, :])
                            nc.sync.dma_start(
                                p3pad[gm:gm + 32, 9:10, 1:33],
                                vp3[96:128, r, 0:1, :])

                # ------------- pooled convs (band bc = it-1) -----------------
                if 1 <= it <= NB:
                    # pooled convs (all 4 strips at once); bf16 in-place
                    # accumulation — all-bf16 unit-stride STT is eligible for
                    # the DVE packed mode, and the bf16 rounding error stays
                    # well inside the tolerance.
                    conv1 = cpool.tile([128, c2n, 128], bf16, tag="c1b")
                    conv2 = cpool.tile([128, c4n, 64], bf16, tag="c2b")
                    conv3 = cpool.tile([128, c8n, 32], bf16, tag="c3b")
                    convs[bc] = (conv1, conv2, conv3)
                    for j in range(9):
                        dy, dx = j // 3, j % 3
                        a1 = p1pad[:, sc // 2 + dy:sc // 2 + dy + c2n,
                                   dx:dx + 128]
                        a2 = p2pad[:, sc // 4 + dy:sc // 4 + dy + c4n,
                                   dx:dx + 64]
                        a3 = p3pad[:, sc // 8 + dy:sc // 8 + dy + c8n,
                                   dx:dx + 32]
                        if j == 0:
                            nc.vector.tensor_scalar_mul(
                                conv1[:], a1, wdwp[:, 0, 0:1])
                            nc.vector.tensor_scalar_mul(
                                conv2[:], a2, wdwp[:, 1, 0:1])
                            nc.vector.tensor_scalar_mul(
                                conv3[:], a3, wdwp[:, 2, 0:1])
                        else:
                            nc.vector.scalar_tensor_tensor(
                                conv1[:], a1, wdwp[:, 0, j:j + 1], conv1[:],
                                AOT.mult, AOT.add)
                            nc.vector.scalar_tensor_tensor(
                                conv2[:], a2, wdwp[:, 1, j:j + 1], conv2[:],
                                AOT.mult, AOT.add)
                            nc.vector.scalar_tensor_tensor(
                                conv3[:], a3, wdwp[:, 2, j:j + 1], conv3[:],
                                AOT.mult, AOT.add)

                # ------------- matmuls + epilogue (band bb = it-2) -----------
                if it >= 2:
                    bb = it - 2
                    sb_, nb_ = BANDS[bb]
                    conv1, conv2, conv3 = convs.pop(bb)
                    x0b = x0s.pop(bb)
                    xbb = xbands.pop(bb)
                    # multiply-engine split: DVE helps at the tail when it has
                    # no more pooling/conv work; GPSIMD carries the rest.
                    dve_rs = (0, 1) if bb == NB - 1 else \
                        ((0,) if bb >= 1 else ())
                    mts = [mpool.tile([128, nb_, 256], bf16, tag="mchunk",
                                      name=f"mt_{bb}_{r}")
                           for r in range(4)]
                    for i in range(nb_ // 2):
                        pss = [pspool.tile([128, 2, 256], f32, tag="pschunk",
                                           name=f"ps_{bb}_{i}_{r}")
                               for r in range(4)]
                        # taps first (need only x0), conv groups last — so
                        # the PE starts a band's chunks while DVE finishes
                        # that band's pooled convs.
                        for tt in range(12):
                            t = tt + 3 if tt < 9 else tt - 9
                            for r in range(4):
                                g0 = 32 * r
                                if t < 3:
                                    lhsT = wall[g0:g0 + 32, 9 + t, :]
                                    if t == 0:
                                        rhs = conv1[g0:g0 + 32, i, :] \
                                            .unsqueeze(1).unsqueeze(3) \
                                            .broadcast_to([32, 2, 128, 2])
                                    elif t == 1:
                                        i2 = (sb_ // 2 + i) // 2 - sb_ // 4
                                        rhs = conv2[g0:g0 + 32, i2, :] \
                                            .unsqueeze(1).unsqueeze(3) \
                                            .broadcast_to([32, 2, 64, 4])
                                    else:
                                        i3 = (sb_ // 2 + i) // 4 - sb_ // 8
                                        rhs = conv3[g0:g0 + 32, i3, :] \
                                            .unsqueeze(1).unsqueeze(3) \
                                            .broadcast_to([32, 2, 32, 8])
                                else:
                                    j = t - 3
                                    dy, dx = j // 3, j % 3
                                    lhsT = wall[g0:g0 + 32, j, :]
                                    rhs = x0b[g0:g0 + 32,
                                              2 * i + dy:2 * i + dy + 2,
                                              dx:dx + 256]
                                nc.tensor.matmul(
                                    pss[r][:], lhsT, rhs,
                                    start=(tt == 0), stop=(tt == 11),
                                    tile_position=(g0, 0))
                        for r in range(4):
                            gt = gpool.tile([128, 2, 256], bf16, tag="gchunk")
                            nc.scalar.activation(
                                gt[:], pss[r][:], act_func, bias=btot[:, 0:1])
                            ms = mts[r][:, 2 * i:2 * i + 2, :]
                            xs = xbb[:, r, 2 * i:2 * i + 2, :]
                            if r in dve_rs:
                                nc.vector.tensor_tensor(
                                    ms, gt[:], xs, AOT.mult)
                            else:
                                nc.gpsimd.tensor_mul(ms, gt[:], xs)
                    for r in range(4):
                        nc.scalar.dma_start(
                            out_d[:, 64 * r + sb_: 64 * r + sb_ + nb_, :],
                            mts[r][:])

    nc.compile()
    _PROGRAM_CACHE[key] = nc
    return nc


def make_in_maps(x, w_dw, b_dw, w_f1, b_f1, w_f2, b_f2, w_f3, b_f3):
    import ml_dtypes
    bf = ml_dtypes.bfloat16
    x = np.asarray(x)
    B = x.shape[0]
    W_all, b_tot, wdwp = fold_weights(
        np.asarray(w_dw), np.asarray(b_dw), np.asarray(w_f1), np.asarray(b_f1),
        np.asarray(w_f2), np.asarray(b_f2), np.asarray(w_f3), np.asarray(b_f3))
    wall_b = np.ascontiguousarray(W_all.astype(bf))
    in_maps = [{"x": np.ascontiguousarray(x[i].astype(bf)),
                "wall": wall_b, "btot": b_tot, "wdwp": wdwp}
               for i in range(B)]
    return in_maps


def kernel(x, w_dw, b_dw, w_f1, b_f1, w_f2, b_f2, w_f3, b_f3):
    from concourse.bass_utils import run_bass_kernel_spmd

    x = np.asarray(x)
    B = x.shape[0]
    in_maps = make_in_maps(x, w_dw, b_dw, w_f1, b_f1, w_f2, b_f2, w_f3, b_f3)
    nc = build_program("Gelu")
    res = run_bass_kernel_spmd(nc, in_maps, list(range(B)))
    out = np.stack([res.results[i]["out"] for i in range(B)], axis=0)
    return out.astype(np.float32)
